# revision 1
# baseline (speedup 1.0000x reference)
"""GAT layer (gnn_message_passing) on 8 trn2 NeuronCores.

Strategy (dst-sharded, no collectives):
- Each core owns a contiguous 1/8 slice of target nodes; host buckets edges by
  dst core. Within a core, owned nodes are sorted by in-degree (descending) and
  grouped into 128-node windows; node -> SBUF partition, its in-edges occupy
  "slot columns" t=0..deg-1 of that partition (degree sorting makes the
  per-window column count ~= mean degree, tiny padding).
- Per edge slot, a 1280B row [xp[2j] | xp[2j+1] | a_s[2j] | a_s[2j+1] | pad]
  is fetched with SWDGE dma_gather (idx = perm_pos(src)>>1 fits int16; the
  pair covers all 50000 nodes). Table built on device in pass-0:
  xp = x @ W_lin.T, a_s = x @ fold(W_lin, w_s). Parity + slot-validity are
  folded into host sel_lo/sel_hi masks.
- Attention logits: a_e from slotted edge_attr (DVE grouped reduce with a
  replicated folded C), a_t + all scalar biases from pass-0 (x @ D_ext) as a
  per-node column. leaky-relu on DVE (scalar_tensor_tensor), exp on ACT.
  Softmax max-subtraction dropped: logits are O(1), softmax shift-invariant.
- msg = expv * xs into an rhs buffer (expv appended as 4 extra cols); window
  numerator+denominator = ONE DVE tensor_reduce(axis=XY) over the slot dims.
  Residual x @ W_res.T + bias via ones-row-extended matmul (PE, PSUM).
  out = num/denom + res.
"""
import os
import sys
from contextlib import ExitStack

sys.path.insert(0, "/opt/trn_rl_repo")

import numpy as np

N, E = 50000, 1600000
IN_F, EDGE_F, HEADS, OUT_F = 64, 16, 4, 32
NEG_SLOPE = 0.2
NCORES = 8
NODES_PC = N // NCORES            # 6250
NW = (NODES_PC + 127) // 128      # 49 windows/core
WNODES = NW * 128                 # 6272 (last window partially real)
TC_TILES = 14                     # gather-chunk size in 128-slot tiles
ROWF = 320                        # gather-table row: 256 xp-pair + 8 a_s + 56 pad


def _host_preprocess(x, edge_index, edge_attr, W_lin, w_s, b_s, w_t, b_t,
                     W_edge, w_e, b_e, W_res, bias):
    """Pure index/layout work + weight folding. Returns (common, per_core)."""
    src = edge_index[0].astype(np.int64)
    dst = edge_index[1].astype(np.int64)
    deg = np.bincount(dst, minlength=N)

    # ---- weight folding (weights only; standard operator fusion) ----
    wlinT = np.ascontiguousarray(W_lin.T)                      # [64, 128]
    C = (W_edge.reshape(HEADS, OUT_F, EDGE_F) * w_e[None, :, None]).sum(1)  # [4,16]
    crep = np.tile(C.reshape(-1)[None, :], (128, 1)).astype(np.float32)    # [128,64]
    D = (W_lin.reshape(HEADS, OUT_F, IN_F) * w_t[None, :, None]).sum(1).T  # [64,4]
    b_total = float(b_s) + float(b_t) + float(b_e)
    dext = np.vstack([D, np.full((1, HEADS), b_total, np.float32)]).astype(np.float32)
    Dws = (W_lin.reshape(HEADS, OUT_F, IN_F) * w_s[None, :, None]).sum(1).T  # [64,4]
    dws = Dws.astype(np.float32)
    wrese = np.vstack([W_res.T, bias[None, :]]).astype(np.float32)         # [65,128]

    # ---- per-core schedules (common T_w across cores) ----
    cores = []
    for c in range(NCORES):
        lo = c * NODES_PC
        owned = np.arange(lo, lo + NODES_PC)
        dc = deg[owned]
        order = np.argsort(-dc, kind="stable")
        perm_owned = owned[order]
        degs_sorted = dc[order]
        tw = np.maximum(degs_sorted[::128][:NW], 1).astype(np.int64)
        cores.append(dict(perm_owned=perm_owned, tw=tw))

    T_w = np.max(np.stack([cc["tw"] for cc in cores]), axis=0)  # [NW]
    TOFF = np.concatenate([[0], np.cumsum(T_w)])                # slot col offsets
    SUMT = int(TOFF[-1])

    chunks = []           # (w, t0, t1, icol0)
    icol = 0
    for w in range(NW):
        t = 0
        while t < T_w[w]:
            t1 = min(t + TC_TILES, int(T_w[w]))
            chunks.append((w, t, t1, icol))
            icol += (t1 - t) * 8
            t += t1 - t
    IDXCOLS = icol

    per_core = []
    for c in range(NCORES):
        cc = cores[c]
        perm_owned = cc["perm_owned"]
        rest = np.setdiff1d(np.arange(N), perm_owned, assume_unique=True)
        perm = np.concatenate([perm_owned, rest])
        perm_pos = np.empty(N, np.int64)
        perm_pos[perm] = np.arange(N)

        emask = (dst >= c * NODES_PC) & (dst < (c + 1) * NODES_PC)
        e_ids = np.nonzero(emask)[0]
        d_loc = perm_pos[dst[e_ids]]                 # 0..6249
        eorder = np.argsort(d_loc, kind="stable")
        e_s = e_ids[eorder]
        ds = d_loc[eorder]
        starts = np.searchsorted(ds, np.arange(NODES_PC))
        t_of = np.arange(len(ds)) - starts[ds]
        w_of = ds // 128
        p_of = ds % 128
        col = TOFF[w_of] + t_of

        src_rel = perm_pos[src[e_s]]
        par = (src_rel & 1).astype(np.float32)

        idx_slot = np.zeros((128, SUMT), np.int16)
        sel = np.zeros((2, 128, SUMT), np.float32)
        ea_slot = np.zeros((128, SUMT, EDGE_F), np.float32)
        idx_slot[p_of, col] = (src_rel >> 1).astype(np.int16)
        sel[0, p_of, col] = 1.0 - par
        sel[1, p_of, col] = par
        ea_slot[p_of, col] = edge_attr[e_s]

        idx16 = np.zeros((128, IDXCOLS), np.int16)
        for (w, t0, t1, ic0) in chunks:
            ncol = (t1 - t0) * 8
            flat = idx_slot[:, TOFF[w] + t0: TOFF[w] + t1].T.reshape(-1)
            wrapped = flat.reshape(-1, 16).T
            idx16[:, ic0: ic0 + ncol] = np.tile(wrapped, (8, 1))

        xT_ext = np.empty((IN_F + 1, N), np.float32)
        xT_ext[:IN_F] = x[perm].T
        xT_ext[IN_F] = 1.0

        per_core.append(dict(
            xT=xT_ext,
            idx16=idx16,
            sel=sel,
            ea=ea_slot.reshape(128, SUMT * EDGE_F),
            perm_owned=perm_owned,
        ))

    wlind = np.concatenate([wlinT.astype(np.float32), dws], axis=1)  # [64, 132]
    common = dict(T_w=T_w, TOFF=TOFF, SUMT=SUMT, chunks=chunks, IDXCOLS=IDXCOLS,
                  wlind=wlind, dext=dext, crep=crep, wrese=wrese)
    return common, per_core


def _build_program(common):
    import concourse.bass as bass
    import concourse.tile as tile
    from concourse import bacc, mybir

    f32 = mybir.dt.float32
    i16 = mybir.dt.int16
    AL = mybir.AluOpType
    AX = mybir.AxisListType
    SUMT, IDXCOLS = common["SUMT"], common["IDXCOLS"]
    T_w, TOFF, chunks = common["T_w"], common["TOFF"], common["chunks"]

    nc = bacc.Bacc("TRN2", target_bir_lowering=False, debug=False,
                   num_devices=NCORES, num_swdge_queues=4)

    xT_d = nc.dram_tensor("xT", [IN_F + 1, N], f32, kind="ExternalInput")
    idx_d = nc.dram_tensor("idx16", [128, IDXCOLS], i16, kind="ExternalInput")
    sel_d = nc.dram_tensor("sel", [2, 128, SUMT], f32, kind="ExternalInput")
    ea_d = nc.dram_tensor("ea", [128, SUMT * EDGE_F], f32, kind="ExternalInput")
    wlin_d = nc.dram_tensor("wlind", [IN_F, 132], f32, kind="ExternalInput")
    dext_d = nc.dram_tensor("dext", [IN_F + 1, HEADS], f32, kind="ExternalInput")
    crep_d = nc.dram_tensor("crep", [128, HEADS * EDGE_F], f32, kind="ExternalInput")
    wrese_d = nc.dram_tensor("wrese", [IN_F + 1, 128], f32, kind="ExternalInput")
    out_d = nc.dram_tensor("out", [WNODES, 128], f32, kind="ExternalOutput")

    with tile.TileContext(nc) as tc, ExitStack() as ctx:
        const = ctx.enter_context(tc.tile_pool(name="const", bufs=1))
        dramp = ctx.enter_context(tc.tile_pool(name="dram", bufs=1, space="DRAM"))
        xp_t = dramp.tile([N // 2, ROWF], f32)

        wlint = const.tile([IN_F, 132], f32)
        nc.sync.dma_start(wlint[:], wlin_d.ap())
        dext_t = const.tile([IN_F + 1, HEADS], f32)
        nc.sync.dma_start(dext_t[:], dext_d.ap())
        crep_t = const.tile([128, HEADS * EDGE_F], f32)
        nc.sync.dma_start(crep_t[:], crep_d.ap())
        wrese_t = const.tile([IN_F + 1, 128], f32)
        nc.sync.dma_start(wrese_t[:], wrese_d.ap())
        xTown = const.tile([IN_F + 1, WNODES], f32)
        nc.sync.dma_start(xTown[:], xT_d.ap()[:, 0:WNODES])
        selL = const.tile([128, SUMT], f32)
        nc.sync.dma_start(selL[:], sel_d.ap()[0])
        selH = const.tile([128, SUMT], f32)
        nc.sync.dma_start(selH[:], sel_d.ap()[1])
        atb = const.tile([128, NW * HEADS], f32)

        # ---- pass-0: gather table ([25000, 320] pair rows) + a_t columns ----
        NBLK = (N + 127) // 128          # 391 node blocks of 128
        GB = 8                           # blocks per batched table write
        SLABW = 12544                    # 98 blocks per slab (slab-aligned groups)
        with tc.tile_pool(name="p0slab", bufs=2) as slabp, \
             tc.tile_pool(name="p0", bufs=3) as p0, \
             tc.tile_pool(name="p0ps", bufs=4, space="PSUM") as p0ps:
            xp_flat = xp_t[:]            # [25000, 320]
            nslab = (N + SLABW - 1) // SLABW
            for sl in range(nslab):
                c0 = sl * SLABW
                cw = min(SLABW, N - c0)
                slab = slabp.tile([IN_F, SLABW], f32, tag="slab")
                nc.sync.dma_start(slab[:, :cw], xT_d.ap()[0:IN_F, c0:c0 + cw])
                b0 = c0 // 128
                bn = (cw + 127) // 128
                for bg in range(b0, b0 + bn, GB):
                    gn = min(GB, b0 + bn - bg)
                    stage = p0.tile([128, GB * 132], f32, tag="stage")
                    for k in range(gn):
                        b = bg + k
                        nb = min(128, N - b * 128)
                        lo = b * 128 - c0
                        if nb < 128:
                            nc.vector.memset(stage[:, k * 132:(k + 1) * 132], 0.0)
                        ps = p0ps.tile([128, 132], f32, tag="ps")
                        nc.tensor.matmul(ps[:nb, :], slab[:, lo:lo + nb],
                                         wlint[:], start=True, stop=True)
                        nc.scalar.copy(stage[:nb, k * 132:(k + 1) * 132], ps[:nb, :])
                    gfull = gn
                    if bg + gn == NBLK and N % 128 != 0:
                        gfull = gn - 1
                    for par in range(2):
                        src = stage[:].rearrange("(r a) c -> a r c", a=2)[par] \
                                      .rearrange("r (k c) -> r k c", c=132)
                        if gfull > 0:
                            dst_xp = xp_flat[64 * bg: 64 * (bg + gfull),
                                             128 * par: 128 * par + 128] \
                                .rearrange("(k r) f -> r k f", k=gfull)
                            nc.sync.dma_start(dst_xp, src[:, :gfull, 0:128])
                            dst_as = xp_flat[64 * bg: 64 * (bg + gfull),
                                             256 + HEADS * par: 256 + HEADS * (par + 1)] \
                                .rearrange("(k r) h -> r k h", k=gfull)
                            nc.sync.dma_start(dst_as, src[:, :gfull, 128:132])
                        if gfull < gn:
                            b = bg + gfull
                            rows = (N - b * 128) // 2     # pair rows in partial block
                            r0 = 64 * b
                            nc.sync.dma_start(
                                xp_flat[r0: r0 + rows, 128 * par: 128 * par + 128],
                                src[:rows, gfull, 0:128])
                            nc.sync.dma_start(
                                xp_flat[r0: r0 + rows,
                                        256 + HEADS * par: 256 + HEADS * (par + 1)],
                                src[:rows, gfull, 128:132])
            for w in range(NW):
                ps2 = p0ps.tile([128, HEADS], f32, tag="ps2")
                nc.tensor.matmul(ps2[:], xTown[:, w * 128:(w + 1) * 128], dext_t[:],
                                 start=True, stop=True)
                nc.scalar.copy(atb[:, w * HEADS:(w + 1) * HEADS], ps2[:])

        # ---- main loop ----
        with tc.tile_pool(name="xsp", bufs=3) as xsp, \
             tc.tile_pool(name="eap", bufs=4) as eap, \
             tc.tile_pool(name="idxp", bufs=4) as idxp, \
             tc.tile_pool(name="scr", bufs=2) as scr, \
             tc.tile_pool(name="sml", bufs=3) as sml, \
             tc.tile_pool(name="rhsp", bufs=2) as rhsp, \
             tc.tile_pool(name="nap", bufs=2) as nap, \
             tc.tile_pool(name="outp", bufs=3) as outp, \
             tc.tile_pool(name="mps", bufs=2, space="PSUM") as mps:

            qrr = 0
            wchunks = {}
            for ch in chunks:
                wchunks.setdefault(ch[0], []).append(ch)

            for w in range(NW):
                res_ps = mps.tile([128, 128], f32, tag="res")
                nc.tensor.matmul(res_ps[:], xTown[:, w * 128:(w + 1) * 128],
                                 wrese_t[:], start=True, stop=True)
                num_acc = nap.tile([128, 132], f32, tag="num")
                first = True
                for (_, t0, t1, ic0) in wchunks[w]:
                    tcn = t1 - t0
                    nidx = tcn * 128
                    scol = int(TOFF[w]) + t0

                    idxc = idxp.tile([128, TC_TILES * 8], i16, tag="idxc")
                    nc.sync.dma_start(idxc[:, :tcn * 8], idx_d.ap()[:, ic0: ic0 + tcn * 8])
                    xs = xsp.tile([128, TC_TILES, ROWF], f32, tag="xs")
                    nsub = min(4, tcn)
                    base = tcn // nsub
                    extra = tcn % nsub
                    tpos = 0
                    for si in range(nsub):
                        stn = base + (1 if si < extra else 0)
                        if stn == 0:
                            continue
                        nc.gpsimd.dma_gather(
                            xs[:, tpos:tpos + stn, :], xp_t[:],
                            idxc[:, tpos * 8:(tpos + stn) * 8],
                            stn * 128, stn * 128, ROWF, single_packet=False,
                            queue_num=qrr % 4)
                        qrr += 1
                        tpos += stn

                    eat = eap.tile([128, TC_TILES * EDGE_F], f32, tag="eat")
                    nc.sync.dma_start(eat[:, :tcn * EDGE_F],
                                      ea_d.ap()[:, scol * EDGE_F: (scol + tcn) * EDGE_F])

                    # a_e: grouped reduce of ea * C
                    prode = scr.tile([128, TC_TILES * HEADS * EDGE_F], f32, tag="prode")
                    ea_bc = eat[:, :tcn * EDGE_F] \
                        .rearrange("p (t k) -> p t k", t=tcn) \
                        .rearrange("p t (a k) -> p t a k", a=1) \
                        .broadcast_to([128, tcn, HEADS, EDGE_F])
                    crep_bc = crep_t[:].rearrange("p (a f) -> p a f", a=1) \
                        .broadcast_to([128, tcn, HEADS * EDGE_F]) \
                        .rearrange("p t (h k) -> p t h k", h=HEADS)
                    prode_v = prode[:, :tcn * HEADS * EDGE_F] \
                        .rearrange("p (t h k) -> p t h k", t=tcn, h=HEADS)
                    prode_g = prode[:, :tcn * HEADS * EDGE_F] \
                        .rearrange("p (g x) -> p g x", x=EDGE_F)
                    ze = sml.tile([128, TC_TILES * HEADS], f32, tag="ze")
                    nc.vector.tensor_tensor(prode_v, ea_bc, crep_bc, op=AL.mult)
                    nc.vector.tensor_reduce(ze[:, :tcn * HEADS], prode_g,
                                            axis=AX.X, op=AL.add)

                    selLb = selL[:, scol: scol + tcn] \
                        .rearrange("p (t a) -> p t a", a=1).broadcast_to([128, tcn, HEADS])
                    selHb = selH[:, scol: scol + tcn] \
                        .rearrange("p (t a) -> p t a", a=1).broadcast_to([128, tcn, HEADS])
                    atbb = atb[:, w * HEADS:(w + 1) * HEADS] \
                        .rearrange("p (a h) -> p a h", a=1).broadcast_to([128, tcn, HEADS])

                    nh = tcn * HEADS
                    # u = as_lo*selL + as_hi*selH + ze + atb  (a_s slices ride the rows)
                    as_lo = xs[:, :tcn, 256:256 + HEADS]
                    as_hi = xs[:, :tcn, 256 + HEADS:256 + 2 * HEADS]
                    t1t = sml.tile([128, TC_TILES * HEADS], f32, tag="t1")
                    t1v = t1t[:, :nh].rearrange("p (t h) -> p t h", t=tcn)
                    nc.vector.tensor_tensor(t1v, as_lo, selLb, op=AL.mult)
                    t2t = sml.tile([128, TC_TILES * HEADS], f32, tag="t2")
                    t2v = t2t[:, :nh].rearrange("p (t h) -> p t h", t=tcn)
                    nc.vector.tensor_tensor(t2v, as_hi, selHb, op=AL.mult)
                    u = sml.tile([128, TC_TILES * HEADS], f32, tag="u")
                    u_v = u[:, :nh].rearrange("p (t h) -> p t h", t=tcn)
                    ze_v = ze[:, :nh].rearrange("p (t h) -> p t h", t=tcn)
                    nc.vector.tensor_tensor(u_v, t1v, t2v, op=AL.add)
                    nc.vector.tensor_tensor(u_v, u_v, ze_v, op=AL.add)
                    nc.vector.tensor_tensor(u_v, u_v, atbb, op=AL.add)
                    lr = sml.tile([128, TC_TILES * HEADS], f32, tag="lr")
                    nc.vector.scalar_tensor_tensor(lr[:, :nh], u[:, :nh], NEG_SLOPE,
                                                   u[:, :nh], op0=AL.mult, op1=AL.max)
                    ev = sml.tile([128, TC_TILES * HEADS], f32, tag="ev")
                    nc.scalar.activation(ev[:, :nh], lr[:, :nh],
                                         mybir.ActivationFunctionType.Exp)
                    ev_v = ev[:, :nh].rearrange("p (t h) -> p t h", t=tcn)

                    rhs = rhsp.tile([128, TC_TILES, 2, 132], f32, tag="rhs")
                    evlo = rhs[:, :tcn, 0, 128:132]
                    evhi = rhs[:, :tcn, 1, 128:132]
                    nc.vector.tensor_tensor(evlo, ev_v, selLb, op=AL.mult)
                    nc.vector.tensor_tensor(evhi, ev_v, selHb, op=AL.mult)
                    evlo_bc = evlo.rearrange("p t (h a) -> p t h a", a=1) \
                                  .broadcast_to([128, tcn, HEADS, OUT_F])
                    evhi_bc = evhi.rearrange("p t (h a) -> p t h a", a=1) \
                                  .broadcast_to([128, tcn, HEADS, OUT_F])
                    msg_lo = rhs[:, :tcn, 0, 0:128].rearrange("p t (h f) -> p t h f", h=HEADS)
                    msg_hi = rhs[:, :tcn, 1, 0:128].rearrange("p t (h f) -> p t h f", h=HEADS)
                    xs_lo4 = xs[:, :tcn, 0:128].rearrange("p t (h f) -> p t h f", h=HEADS)
                    xs_hi4 = xs[:, :tcn, 128:256].rearrange("p t (h f) -> p t h f", h=HEADS)
                    nc.vector.tensor_tensor(msg_lo, xs_lo4, evlo_bc, op=AL.mult)
                    nc.vector.tensor_tensor(msg_hi, xs_hi4, evhi_bc, op=AL.mult)

                    # num += sum over (t, half): contiguous pairwise fold
                    flat = rhs[:].rearrange("p t h f -> p (t h) f")   # [128, 2*TC, 132]
                    n = 2 * tcn
                    while n > 1:
                        k = n // 2
                        nc.vector.tensor_tensor(flat[:, 0:k, :], flat[:, 0:k, :],
                                                flat[:, n - k:n, :], op=AL.add)
                        n -= k
                    if first:
                        nc.vector.tensor_copy(num_acc[:], flat[:, 0, :])
                        first = False
                    else:
                        nc.vector.tensor_tensor(num_acc[:], num_acc[:], flat[:, 0, :],
                                                op=AL.add)

                # ---- window close ----
                dn = outp.tile([128, HEADS], f32, tag="dn")
                nc.vector.tensor_scalar_max(dn[:], num_acc[:, 128:132], 1e-30)
                rec = outp.tile([128, HEADS], f32, tag="rec")
                nc.vector.reciprocal(rec[:], dn[:])
                outw = outp.tile([128, 128], f32, tag="outw")
                outw_v = outw[:].rearrange("p (h f) -> p h f", h=HEADS)
                num_v = num_acc[:, 0:128].rearrange("p (h f) -> p h f", h=HEADS)
                rec_bc = rec[:].rearrange("p (h a) -> p h a", a=1) \
                               .broadcast_to([128, HEADS, OUT_F])
                nc.vector.tensor_tensor(outw_v, num_v, rec_bc, op=AL.mult)
                out2 = outp.tile([128, 128], f32, tag="out2")
                nc.vector.tensor_tensor(out2[:], outw[:], res_ps[:], op=AL.add)
                nc.sync.dma_start(out_d.ap()[w * 128:(w + 1) * 128, :], out2[:])

    nc.compile()
    return nc


def kernel(**inputs):
    from concourse.bass_utils import run_bass_kernel_spmd

    args = {k: np.asarray(v) for k, v in inputs.items()}
    common, per_core = _host_preprocess(
        args["x"], args["edge_index"], args["edge_attr"], args["W_lin"],
        args["w_s"], args["b_s"], args["w_t"], args["b_t"], args["W_edge"],
        args["w_e"], args["b_e"], args["W_res"], args["bias"])

    nc = _build_program(common)

    in_maps = []
    for c in range(NCORES):
        pc = per_core[c]
        in_maps.append({
            "xT": pc["xT"], "idx16": pc["idx16"], "sel": pc["sel"], "ea": pc["ea"],
            "wlind": common["wlind"], "dext": common["dext"],
            "crep": common["crep"], "wrese": common["wrese"],
        })

    res = run_bass_kernel_spmd(nc, in_maps, list(range(NCORES)),
                               trace=bool(os.environ.get("GAT_TRACE")),
                               tmpdir=os.environ.get("GAT_TMPDIR"))
    if os.environ.get("GAT_TRACE"):
        print(f"HW exec time: {res.exec_time_ns} ns")

    out = np.empty((N, HEADS * OUT_F), np.float32)
    for c in range(NCORES):
        out[per_core[c]["perm_owned"]] = res.results[c]["out"][:NODES_PC]
    return out



# revision 9
# speedup vs baseline: 1.4067x; 1.4067x over previous
"""GAT layer (gnn_message_passing) on 8 trn2 NeuronCores.

Strategy (dst-sharded, no collectives), v2 "two-pass single-row" design:

- Each core owns a contiguous 1/8 slice of target nodes. A per-core node
  permutation assigns every node a table row: owned nodes first, then the
  rest; rows 32767 and 50001 are all-zero pad rows. Pass-0 builds a DRAM
  gather table [50002, 256] fp16 where row r = [xp (f-major,h-minor) 128 |
  ones 4 | a_s 4 | zero pad]: xp = x @ W_lin.T, a_s = x @ fold(W_lin, w_s),
  the ones come from an appended 1.0 input row. fp16 rows are 512 B so a
  SWDGE dma_gather descriptor (int16 idx, <=32768 rows per table half)
  fetches ONE node per edge slot - no pair/parity tricks.

- Edges are split by src table row into grid A (row < 32768) and grid B.
  Each grid gets its own pass with its own node arrangement (owned nodes
  sorted by that grid's in-degree, 128-node windows, slot columns =
  per-window max degree -> ~3% padding). Pass A accumulates per-node
  [num | den] partials into DRAM; pass B re-gathers the 128 partials per
  window (one tiny dma_gather) and finishes: divide + residual + store.

- Per window: gather xs [128, T, 256]; logits u = a_s(slot) + ze + atb
  (ze = a_e precomputed for ALL slots during pass-0 from slotted
  edge_attr; atb = a_t + all biases from x @ D_ext); leaky-relu; exp on
  ACT. msg+denominator in ONE fp16 2x DVE mult: rhs[:, t, 0:132] =
  xs[:, t, 0:132] * ev broadcast over the [33, 4] (group, head) view -
  the 'ones' columns turn into the softmax denominator. Pairwise fp16
  fold over slots, then (pass B) + gathered partial, divide, residual
  (PE matmul), store. All heavy DVE ops are fp16 with packed last dims
  to hit the 2x DVE mode.
"""
import os
import sys
from contextlib import ExitStack

sys.path.insert(0, "/opt/trn_rl_repo")

import numpy as np

N, E = 50000, 1600000
IN_F, EDGE_F, HEADS, OUT_F = 64, 16, 4, 32
NEG_SLOPE = 0.2
NCORES = 8
NODES_PC = N // NCORES            # 6250
NW = (NODES_PC + 127) // 128      # 49 windows/core
WNODES = NW * 128                 # 6272
TROWS = N + 2                     # 50002 table rows (2 pad rows)
TROW = 256                        # fp16 elems per table row (512 B)
SPLIT = 32768
PAD_A = 32767                     # pad row in table half A
PAD_B_LOCAL = N + 1 - SPLIT       # pad row 50001, local idx in half B
ZCHUNK = 256                      # ze precompute mega-chunk columns

# device column j (j < 128) holds logical output column (j%4)*32 + j//4
COLIDX = (np.arange(128) % HEADS) * OUT_F + np.arange(128) // HEADS


def _wrap_idx(idx_slot, cols):
    """[128, ncols] slot idx -> SWDGE wrapped [128, ncols*8] int16."""
    flat = idx_slot[:, cols].T.reshape(-1)          # col-major (t, p)
    wrapped = flat.reshape(-1, 16).T                # [16, n/16]
    return np.tile(wrapped, (8, 1)).astype(np.int16)


def _host_preprocess(x, edge_index, edge_attr, W_lin, w_s, b_s, w_t, b_t,
                     W_edge, w_e, b_e, W_res, bias):
    """Pure index/layout work + weight folding. Returns (common, per_core)."""
    f16 = np.float16
    src = edge_index[0].astype(np.int64)
    dst = edge_index[1].astype(np.int64)

    # ---- weight folding (weights only; standard operator fusion) ----
    wlin_perm = W_lin[COLIDX, :].T                              # [64, 128]
    Dws = (W_lin.reshape(HEADS, OUT_F, IN_F) * w_s[None, :, None]).sum(1).T
    wlin_ext = np.zeros((IN_F + 1, 136), np.float32)
    wlin_ext[:IN_F, 0:128] = wlin_perm
    wlin_ext[IN_F, 128:132] = 1.0
    wlin_ext[:IN_F, 132:136] = Dws

    D = (W_lin.reshape(HEADS, OUT_F, IN_F) * w_t[None, :, None]).sum(1).T
    b_total = float(b_s) + float(b_t) + float(b_e)
    dext = np.vstack([D, np.full((1, HEADS), b_total, np.float32)])

    C = (W_edge.reshape(HEADS, OUT_F, EDGE_F) * w_e[None, :, None]).sum(1)
    crep = np.tile(C.reshape(-1)[None, :], (128, 1))            # [128, 64]

    wrese = np.zeros((IN_F + 1, 128), np.float32)
    wrese[:IN_F] = W_res[COLIDX, :].T
    wrese[IN_F] = bias[COLIDX]

    per_core = []
    TA = np.zeros((NCORES, NW), np.int64)
    TB = np.zeros((NCORES, NW), np.int64)
    pre = []
    for c in range(NCORES):
        lo = c * NODES_PC
        owned = np.arange(lo, lo + NODES_PC)
        emask = (dst >= lo) & (dst < lo + NODES_PC)
        e_ids = np.nonzero(emask)[0]
        dloc = dst[e_ids] - lo                       # 0..6249 (unsorted)

        # temp row map (owned order irrelevant for the A/B split: all < 32768)
        rest = np.setdiff1d(np.arange(N), owned, assume_unique=True)
        n_lowrest = SPLIT - 1 - NODES_PC
        isB_node = np.zeros(N, bool)
        isB_node[rest[n_lowrest:]] = True
        isB = isB_node[src[e_ids]]

        degA = np.bincount(dloc[~isB], minlength=NODES_PC)
        degB = np.bincount(dloc[isB], minlength=NODES_PC)
        ordA = np.argsort(-degA, kind="stable")      # A-arrangement
        ordB = np.argsort(-degB, kind="stable")      # B-arrangement
        for w in range(NW):
            TA[c, w] = degA[ordA][w * 128:(w + 1) * 128].max()
            TB[c, w] = degB[ordB][w * 128:(w + 1) * 128].max()
        pre.append(dict(owned=owned, e_ids=e_ids, dloc=dloc, isB=isB,
                        degA=degA, degB=degB, ordA=ordA, ordB=ordB,
                        rest=rest, n_lowrest=n_lowrest))

    TAw = np.maximum(TA.max(axis=0), 1)
    TBw = np.maximum(TB.max(axis=0), 1)
    AOFF = np.concatenate([[0], np.cumsum(TAw)])     # ze/ea col offsets
    BOFF = np.concatenate([[0], np.cumsum(TBw)])
    SUMA, SUMB = int(AOFF[-1]), int(BOFF[-1])
    TMAX = int(max(TAw.max(), TBw.max()))

    for c in range(NCORES):
        p = pre[c]
        owned, e_ids, dloc, isB = p["owned"], p["e_ids"], p["dloc"], p["isB"]
        ordA, ordB = p["ordA"], p["ordB"]
        # table rows: 0..6249 = A-arrangement owned; pad rows 32767, 50001
        perm_pos = np.empty(N, np.int64)
        perm_pos[owned[ordA]] = np.arange(NODES_PC)
        nl = p["n_lowrest"]
        perm_pos[p["rest"][:nl]] = NODES_PC + np.arange(nl)
        perm_pos[p["rest"][nl:]] = SPLIT + np.arange(len(p["rest"]) - nl)

        xT = np.zeros((IN_F + 1, TROWS), np.float32)
        xT[:IN_F, perm_pos] = x.T
        xT[IN_F, perm_pos] = 1.0                     # pad cols stay 0

        # B-arrangement node features for atb_B / residual
        xTB = np.zeros((IN_F + 1, WNODES), np.float32)
        xTB[:IN_F, :NODES_PC] = x[owned[ordB]].T
        xTB[IN_F, :NODES_PC] = 1.0

        # partial-gather idx: B-window row (w,pp) -> A-row of same node
        arow_of_node = np.empty(NODES_PC, np.int64)
        arow_of_node[ordA] = np.arange(NODES_PC)
        pg_rows = arow_of_node[ordB]
        pgidx_slot = np.zeros((128, NW), np.int64)
        for w in range(NW):
            n0 = w * 128
            n1 = min(n0 + 128, NODES_PC)
            pgidx_slot[0:n1 - n0, w] = pg_rows[n0:n1]
            if n1 - n0 < 128:                        # fake rows -> row 6250+
                pgidx_slot[n1 - n0:, w] = np.arange(n1 - n0, 128) + n0
        pgidx = np.concatenate(
            [_wrap_idx(pgidx_slot, [w]) for w in range(NW)], axis=1)

        per_grid = {}
        for g, (ordG, Tw, OFF, SUMT) in (
                ("A", (ordA, TAw, AOFF, SUMA)),
                ("B", (ordB, TBw, BOFF, SUMB))):
            sel = isB if g == "B" else ~isB
            eg = e_ids[sel]
            # position of dst node in this arrangement
            pos_of = np.empty(NODES_PC, np.int64)
            pos_of[ordG] = np.arange(NODES_PC)
            dpos = pos_of[dloc[sel]]
            order = np.argsort(dpos, kind="stable")
            eg = eg[order]
            ds = dpos[order]
            starts = np.searchsorted(ds, np.arange(NODES_PC))
            t_of = np.arange(len(ds)) - starts[ds]
            w_of = ds // 128
            p_of = ds % 128
            col = OFF[w_of] + t_of

            s_row = perm_pos[src[eg]]
            if g == "B":
                s_row = s_row - SPLIT
                padrow = PAD_B_LOCAL
            else:
                padrow = PAD_A
            idx_slot = np.full((128, SUMT), padrow, np.int64)
            idx_slot[p_of, col] = s_row
            ea_slot = np.zeros((128, SUMT, EDGE_F), np.float32)
            ea_slot[p_of, col] = edge_attr[eg]

            idx16 = np.concatenate(
                [_wrap_idx(idx_slot, list(range(int(OFF[w]), int(OFF[w + 1]))))
                 for w in range(NW)], axis=1)
            per_grid[g] = dict(idx=idx16,
                               ea=ea_slot.reshape(128, SUMT * EDGE_F)
                                         .astype(f16))

        per_core.append(dict(
            xT=xT.astype(f16), xTB=xTB.astype(f16),
            idxA=per_grid["A"]["idx"], idxB=per_grid["B"]["idx"],
            eaA=per_grid["A"]["ea"], eaB=per_grid["B"]["ea"],
            pgidx=pgidx, out_nodes=owned[ordB],
        ))

    common = dict(TAw=TAw, TBw=TBw, AOFF=AOFF, BOFF=BOFF,
                  SUMA=SUMA, SUMB=SUMB, TMAX=TMAX,
                  wlin_ext=wlin_ext.astype(f16), dext=dext.astype(f16),
                  crep=crep.astype(f16), wrese=wrese.astype(f16))
    return common, per_core


def _build_program(common):
    import concourse.bass as bass
    import concourse.tile as tile
    from concourse import bacc, mybir

    f32 = mybir.dt.float32
    f16 = mybir.dt.float16
    i16 = mybir.dt.int16
    AL = mybir.AluOpType
    TAw, TBw = common["TAw"], common["TBw"]
    AOFF, BOFF = common["AOFF"], common["BOFF"]
    SUMA, SUMB = common["SUMA"], common["SUMB"]
    TMAX = common["TMAX"]

    nc = bacc.Bacc("TRN2", target_bir_lowering=False, debug=False,
                   num_devices=NCORES, num_swdge_queues=4)

    xT_d = nc.dram_tensor("xT", [IN_F + 1, TROWS], f16, kind="ExternalInput")
    xTB_d = nc.dram_tensor("xTB", [IN_F + 1, WNODES], f16, kind="ExternalInput")
    idxA_d = nc.dram_tensor("idxA", [128, SUMA * 8], i16, kind="ExternalInput")
    idxB_d = nc.dram_tensor("idxB", [128, SUMB * 8], i16, kind="ExternalInput")
    pgidx_d = nc.dram_tensor("pgidx", [128, NW * 8], i16, kind="ExternalInput")
    eaA_d = nc.dram_tensor("eaA", [128, SUMA * EDGE_F], f16, kind="ExternalInput")
    eaB_d = nc.dram_tensor("eaB", [128, SUMB * EDGE_F], f16, kind="ExternalInput")
    wlin_d = nc.dram_tensor("wlin_ext", [IN_F + 1, 136], f16, kind="ExternalInput")
    dext_d = nc.dram_tensor("dext", [IN_F + 1, HEADS], f16, kind="ExternalInput")
    crep_d = nc.dram_tensor("crep", [128, HEADS * EDGE_F], f16, kind="ExternalInput")
    wrese_d = nc.dram_tensor("wrese", [IN_F + 1, 128], f16, kind="ExternalInput")
    out_d = nc.dram_tensor("out", [WNODES, 128], f32, kind="ExternalOutput")

    with tile.TileContext(nc) as tc, ExitStack() as ctx:
        const = ctx.enter_context(tc.tile_pool(name="const", bufs=1))
        dramp = ctx.enter_context(tc.tile_pool(name="dram", bufs=1, space="DRAM"))
        xp_t = dramp.tile([TROWS, TROW], f16)
        part_t = dramp.tile([WNODES, TROW], f16)

        wlint = const.tile([IN_F + 1, 136], f16)
        nc.sync.dma_start(wlint[:], wlin_d.ap())
        dext_t = const.tile([IN_F + 1, HEADS], f16)
        nc.sync.dma_start(dext_t[:], dext_d.ap())
        crep_t = const.tile([128, HEADS * EDGE_F], f16)
        nc.sync.dma_start(crep_t[:], crep_d.ap())
        wrese_t = const.tile([IN_F + 1, 128], f16)
        nc.sync.dma_start(wrese_t[:], wrese_d.ap())
        xTown = const.tile([IN_F + 1, WNODES], f16)
        nc.sync.dma_start(xTown[:], xT_d.ap()[:, 0:WNODES])
        xTBt = const.tile([IN_F + 1, WNODES], f16)
        nc.sync.dma_start(xTBt[:], xTB_d.ap())
        pgidx_t = const.tile([128, NW * 8], i16)
        nc.sync.dma_start(pgidx_t[:], pgidx_d.ap())
        atbA = const.tile([128, NW * HEADS], f16)
        atbB = const.tile([128, NW * HEADS], f16)
        zeA = const.tile([128, SUMA * HEADS], f16)
        zeB = const.tile([128, SUMB * HEADS], f16)

        # ---- pass-0a: ze (= a_e) for every slot of both grids ----
        with tc.tile_pool(name="zep", bufs=2) as zep:
            for ze_t, ea_d, SUMT in ((zeA, eaA_d, SUMA), (zeB, eaB_d, SUMB)):
                c0 = 0
                while c0 < SUMT:
                    cw = min(ZCHUNK, SUMT - c0)
                    eat = zep.tile([128, ZCHUNK * EDGE_F], f16, tag="eat")
                    nc.sync.dma_start(eat[:, :cw * EDGE_F],
                                      ea_d.ap()[:, c0 * EDGE_F:(c0 + cw) * EDGE_F])
                    prode = zep.tile([128, ZCHUNK * HEADS * EDGE_F], f16,
                                     tag="prode")
                    ea_bc = eat[:, :cw * EDGE_F] \
                        .rearrange("p (t k) -> p t k", t=cw) \
                        .rearrange("p t (a k) -> p t a k", a=1) \
                        .broadcast_to([128, cw, HEADS, EDGE_F])
                    crep_bc = crep_t[:].rearrange("p (a f) -> p a f", a=1) \
                        .broadcast_to([128, cw, HEADS * EDGE_F]) \
                        .rearrange("p t (h k) -> p t h k", h=HEADS)
                    pv = prode[:, :cw * HEADS * EDGE_F] \
                        .rearrange("p (g k) -> p g k", k=EDGE_F)
                    nc.vector.tensor_tensor(
                        pv.rearrange("p (t h) k -> p t h k", h=HEADS),
                        ea_bc, crep_bc, op=AL.mult)
                    nc.vector.tensor_tensor(pv[:, :, 0:8], pv[:, :, 0:8],
                                            pv[:, :, 8:16], op=AL.add)
                    nc.vector.tensor_tensor(pv[:, :, 0:4], pv[:, :, 0:4],
                                            pv[:, :, 4:8], op=AL.add)
                    nc.vector.tensor_tensor(pv[:, :, 0:2], pv[:, :, 0:2],
                                            pv[:, :, 2:4], op=AL.add)
                    zv = ze_t[:, c0 * HEADS:(c0 + cw) * HEADS] \
                        .rearrange("p (g a) -> p g a", a=1)
                    nc.vector.tensor_tensor(zv, pv[:, :, 0:1], pv[:, :, 1:2],
                                            op=AL.add)
                    c0 += cw

        # ---- pass-0b: gather table + atb columns ----
        NBLK = (TROWS + 127) // 128      # 391 row blocks
        GB = 8
        SLABW = 12544
        with tc.tile_pool(name="p0slab", bufs=2) as slabp, \
             tc.tile_pool(name="p0", bufs=3) as p0, \
             tc.tile_pool(name="p0ps", bufs=4, space="PSUM") as p0ps:
            xp_flat = xp_t[:]            # [50002, 256]
            nslab = (TROWS + SLABW - 1) // SLABW
            for sl in range(nslab):
                c0 = sl * SLABW
                cw = min(SLABW, TROWS - c0)
                slab = slabp.tile([IN_F + 1, SLABW], f16, tag="slab")
                nc.sync.dma_start(slab[:, :cw], xT_d.ap()[:, c0:c0 + cw])
                b0 = c0 // 128
                bn = (cw + 127) // 128
                for bg in range(b0, b0 + bn, GB):
                    gn = min(GB, b0 + bn - bg)
                    gfull = gn
                    if bg + gn == NBLK and TROWS % 128 != 0:
                        gfull = gn - 1
                    stage = p0.tile([128, GB * 136], f16, tag="stage")
                    for k in range(gn):
                        b = bg + k
                        nb = min(128, TROWS - b * 128)
                        lo = b * 128 - c0
                        ps = p0ps.tile([128, 136], f32, tag="ps")
                        nc.tensor.matmul(ps[:nb, :], slab[:, lo:lo + nb],
                                         wlint[:], start=True, stop=True)
                        nc.scalar.copy(stage[:nb, k * 136:(k + 1) * 136],
                                       ps[:nb, :])
                    if gfull > 0:
                        dst_xp = xp_flat[128 * bg:128 * (bg + gfull), 0:136] \
                            .rearrange("(k r) f -> r k f", k=gfull)
                        nc.sync.dma_start(
                            dst_xp,
                            stage[:].rearrange("r (k c) -> r k c", c=136)
                                    [:, :gfull, :])
                    if gfull < gn:
                        b = bg + gfull
                        nb = TROWS - b * 128
                        nc.sync.dma_start(
                            xp_flat[128 * b:128 * b + nb, 0:136],
                            stage[:nb, gfull * 136:(gfull + 1) * 136])
        with tc.tile_pool(name="atbps", bufs=4, space="PSUM") as atbps:
            for w in range(NW):
                ps2 = atbps.tile([128, HEADS], f32, tag="ps2")
                nc.tensor.matmul(ps2[:], xTown[:, w * 128:(w + 1) * 128],
                                 dext_t[:], start=True, stop=True)
                nc.scalar.copy(atbA[:, w * HEADS:(w + 1) * HEADS], ps2[:])
                ps3 = atbps.tile([128, HEADS], f32, tag="ps2")
                nc.tensor.matmul(ps3[:], xTBt[:, w * 128:(w + 1) * 128],
                                 dext_t[:], start=True, stop=True)
                nc.scalar.copy(atbB[:, w * HEADS:(w + 1) * HEADS], ps3[:])

        # ---- main: pass A then pass B, one window per step ----
        with tc.tile_pool(name="xsp", bufs=3) as xsp, \
             tc.tile_pool(name="idxp", bufs=4) as idxp, \
             tc.tile_pool(name="rhsp", bufs=2) as rhsp, \
             tc.tile_pool(name="sml", bufs=3) as sml, \
             tc.tile_pool(name="pap", bufs=2) as pap, \
             tc.tile_pool(name="outp", bufs=3) as outp, \
             tc.tile_pool(name="mps", bufs=2, space="PSUM") as mps:

            qrr = 0
            for phase in ("A", "B"):
                Tw = TAw if phase == "A" else TBw
                OFF = AOFF if phase == "A" else BOFF
                idx_d = idxA_d if phase == "A" else idxB_d
                ze_t = zeA if phase == "A" else zeB
                atb = atbA if phase == "A" else atbB
                tab = xp_flat[0:SPLIT, :] if phase == "A" \
                    else xp_flat[SPLIT:TROWS, :]
                for w in range(NW):
                    tw = int(Tw[w])
                    scol = int(OFF[w])
                    idxc = idxp.tile([128, TMAX * 8], i16, tag="idxc")
                    nc.sync.dma_start(idxc[:, :tw * 8],
                                      idx_d.ap()[:, scol * 8:(scol + tw) * 8])
                    xs = xsp.tile([128, TMAX, TROW], f16, tag="xs")
                    nc.gpsimd.dma_gather(
                        xs[:, 0:tw, :], tab, idxc[:, :tw * 8],
                        tw * 128, tw * 128, TROW, single_packet=False,
                        queue_num=qrr % 4)
                    qrr += 1

                    if phase == "B":
                        res_ps = mps.tile([128, 128], f32, tag="res")
                        nc.tensor.matmul(res_ps[:],
                                         xTBt[:, w * 128:(w + 1) * 128],
                                         wrese_t[:], start=True, stop=True)
                        pa = pap.tile([128, TROW], f16, tag="pa")
                        nc.gpsimd.dma_gather(
                            pa[:].rearrange("p (a f) -> p a f", a=1),
                            part_t[:], pgidx_t[:, w * 8:(w + 1) * 8],
                            128, 128, TROW, single_packet=False,
                            queue_num=qrr % 4)
                        qrr += 1

                    nh = tw * HEADS
                    atbb = atb[:, w * HEADS:(w + 1) * HEADS] \
                        .rearrange("p (a h) -> p a h", a=1) \
                        .broadcast_to([128, tw, HEADS])
                    as_v = xs[:, 0:tw, 132:136]
                    t1 = sml.tile([128, TMAX * HEADS], f16, tag="t1")
                    t1v = t1[:, :nh].rearrange("p (t h) -> p t h", t=tw)
                    nc.vector.tensor_tensor(t1v, as_v, atbb, op=AL.add)
                    u = sml.tile([128, TMAX * HEADS], f16, tag="u")
                    nc.vector.tensor_tensor(
                        u[:, :nh], t1[:, :nh],
                        ze_t[:, scol * HEADS:(scol + tw) * HEADS], op=AL.add)
                    lr = sml.tile([128, TMAX * HEADS], f16, tag="lr")
                    nc.vector.scalar_tensor_tensor(
                        lr[:, :nh], u[:, :nh], NEG_SLOPE, u[:, :nh],
                        op0=AL.mult, op1=AL.max)
                    ev = sml.tile([128, TMAX * HEADS], f16, tag="ev")
                    nc.scalar.activation(ev[:, :nh], lr[:, :nh],
                                         mybir.ActivationFunctionType.Exp)

                    rhs = rhsp.tile([128, TMAX, 132], f16, tag="rhs")
                    ev_bc = ev[:, :nh].rearrange("p (t h) -> p t h", t=tw) \
                        .rearrange("p t (a h) -> p t a h", a=1) \
                        .broadcast_to([128, tw, 33, HEADS])
                    xs_v = xs[:, 0:tw, 0:132] \
                        .rearrange("p t (g h) -> p t g h", h=HEADS)
                    rhs_v = rhs[:, 0:tw, :] \
                        .rearrange("p t (g h) -> p t g h", h=HEADS)
                    nc.vector.tensor_tensor(rhs_v, xs_v, ev_bc, op=AL.mult)

                    n = tw
                    while n > 1:
                        k = n // 2
                        nc.vector.tensor_tensor(
                            rhs[:, 0:k, :], rhs[:, 0:k, :],
                            rhs[:, n - k:n, :], op=AL.add)
                        n -= k

                    if phase == "A":
                        nc.sync.dma_start(
                            part_t[w * 128:(w + 1) * 128, 0:132],
                            rhs[:, 0, :])
                    else:
                        tot = outp.tile([128, 132], f16, tag="tot")
                        nc.vector.tensor_tensor(tot[:], rhs[:, 0, :],
                                                pa[:, 0:132], op=AL.add)
                        dn = outp.tile([128, HEADS], f32, tag="dn")
                        nc.scalar.copy(dn[:], tot[:, 128:132])
                        nc.vector.tensor_scalar_max(dn[:], dn[:], 1e-4)
                        rec = outp.tile([128, HEADS], f32, tag="rec")
                        nc.vector.reciprocal(rec[:], dn[:])
                        outw = outp.tile([128, 128], f32, tag="outw")
                        outw_v = outw[:].rearrange("p (g h) -> p g h", h=HEADS)
                        num_v = tot[:, 0:128].rearrange("p (g h) -> p g h",
                                                        h=HEADS)
                        rec_bc = rec[:].rearrange("p (a h) -> p a h", a=1) \
                            .broadcast_to([128, 32, HEADS])
                        nc.vector.tensor_tensor(outw_v, num_v, rec_bc,
                                                op=AL.mult)
                        out2 = outp.tile([128, 128], f32, tag="out2")
                        nc.vector.tensor_tensor(out2[:], outw[:], res_ps[:],
                                                op=AL.add)
                        nc.sync.dma_start(out_d.ap()[w * 128:(w + 1) * 128, :],
                                          out2[:])

    if not os.environ.get("GAT_SKIP_COMPILE"):
        nc.compile()
    return nc


def kernel(**inputs):
    from concourse.bass_utils import run_bass_kernel_spmd

    args = {k: np.asarray(v) for k, v in inputs.items()}
    common, per_core = _host_preprocess(
        args["x"], args["edge_index"], args["edge_attr"], args["W_lin"],
        args["w_s"], args["b_s"], args["w_t"], args["b_t"], args["W_edge"],
        args["w_e"], args["b_e"], args["W_res"], args["bias"])

    nc = _build_program(common)

    in_maps = []
    for c in range(NCORES):
        pc = per_core[c]
        in_maps.append({
            "xT": pc["xT"], "xTB": pc["xTB"],
            "idxA": pc["idxA"], "idxB": pc["idxB"], "pgidx": pc["pgidx"],
            "eaA": pc["eaA"], "eaB": pc["eaB"],
            "wlin_ext": common["wlin_ext"], "dext": common["dext"],
            "crep": common["crep"], "wrese": common["wrese"],
        })

    res = run_bass_kernel_spmd(nc, in_maps, list(range(NCORES)),
                               trace=bool(os.environ.get("GAT_TRACE")),
                               tmpdir=os.environ.get("GAT_TMPDIR"))
    if os.environ.get("GAT_TRACE"):
        print(f"HW exec time: {res.exec_time_ns} ns")

    out = np.empty((N, HEADS * OUT_F), np.float32)
    for c in range(NCORES):
        dev = res.results[c]["out"][:NODES_PC]       # [6250, 128] device cols
        logical = np.empty_like(dev)
        logical[:, COLIDX] = dev                     # device col j -> logical
        out[per_core[c]["out_nodes"]] = logical
    return out
